# revision 1
# baseline (speedup 1.0000x reference)
"""Grouped GEMM (MoE routing) Trainium2 kernel.

Strategy: tensor-parallel shard of the output N dim across 8 NeuronCores.
Every core sees all T=8192 tokens and a 512-wide slice of every expert's
weights, so per-core work is identical regardless of segment sizes and a
single SPMD program (with the segment boundaries baked in as compile-time
constants) runs on all 8 cores.

Per core:  out_t[n, t] = sum_k w_t[e(t), k, n] * a_t[k, t]
  - a_t   : a transposed to [K, T]  (shared by all cores)
  - w_t   : per-core weight slices [E_active, K, 512] (K-major)
  - out_t : [512, T]; host concatenates along N and transposes back.

Matmul mapping: stationary lhsT = w_t tile [k=128, n=128], moving rhs =
a_t tile [k=128, tok<=512], PSUM out [n=128, tok<=512], accumulated over
the 32 k-chunks.  dtype float32r -> full-rate fp32 when moving dim >= 256,
so segments are split into even token pieces of 256..512.
"""

import numpy as np

import concourse.bacc as bacc
import concourse.bass as bass
import concourse.mybir as mybir
import concourse.tile as tile
from concourse.bass_utils import run_bass_kernel_spmd

NC = 8          # NeuronCores
P = 128         # partitions
TB = 512        # max token block (moving free dim, one PSUM bank of fp32)
KOC = 8         # k-chunks per a-tile DMA batch

LAST_RESULT = {}


def _token_blocks(seg_starts, seg_ends):
    """Split each segment into even pieces of <=512 tokens (>=256 when the
    segment allows, keeping float32r at full rate)."""
    blocks = []  # (tstart, tlen, active_expert_idx)
    for widx, (s, t) in enumerate(zip(seg_starts, seg_ends)):
        ln = t - s
        npieces = max(1, -(-ln // TB))
        base, rem = divmod(ln, npieces)
        p = s
        for i in range(npieces):
            L = base + (1 if i < rem else 0)
            if L > 0:
                blocks.append((p, L, widx))
                p += L
    return blocks


def _build_program(T, K, NS, EA, blocks):
    f32 = mybir.dt.float32
    f32r = mybir.dt.float32r
    KO = K // P
    NB = NS // P
    koc_n = min(KOC, KO)

    nc = bacc.Bacc(None, target_bir_lowering=False)
    at = nc.declare_dram_parameter("at", [KO, P, T], f32r, isOutput=False)
    wt = nc.declare_dram_parameter("wt", [EA, KO, P, NS], f32r, isOutput=False)
    ot = nc.declare_dram_parameter("ot", [NB, P, T], f32, isOutput=True)

    with tile.TileContext(nc) as tc:
        with (
            tc.tile_pool(name="wpool", bufs=2) as wpool,
            tc.tile_pool(name="apool", bufs=2) as apool,
            tc.tile_pool(name="opool", bufs=2) as opool,
            tc.tile_pool(name="psum", bufs=8, space=bass.MemorySpace.PSUM) as psum_pool,
        ):
            cur_widx = -1
            w_tile = None
            for (ts, L, widx) in blocks:
                # f32r matmuls need an even moving size: widen odd blocks by
                # one token for compute, write back only the real L columns.
                Lc = L + (L % 2)
                tsc = ts if ts + Lc <= T else ts - 1
                off = ts - tsc
                if widx != cur_widx:
                    w_tile = wpool.tile([P, KO, NS], f32r, tag="w", name="w_tile")
                    # one 8MB DMA: src (ko, kp, n) -> dst (kp, ko, n)
                    nc.sync.dma_start(
                        out=w_tile[:, :, :],
                        in_=wt[widx].transpose([1, 0, 2]),
                    )
                    cur_widx = widx
                ptiles = [psum_pool.tile([P, Lc], f32, tag="ps", name=f"ps{nb}",
                                         padded_shape=[P, TB])
                          for nb in range(NB)]
                for koc in range(KO // koc_n):
                    a_tile = apool.tile([P, koc_n, Lc], f32r, tag="a", name="a_tile",
                                        padded_shape=[P, koc_n, TB])
                    nc.sync.dma_start(
                        out=a_tile[:, :, :],
                        in_=at[koc * koc_n:(koc + 1) * koc_n, :, tsc:tsc + Lc]
                        .transpose([1, 0, 2]),
                    )
                    for koi in range(koc_n):
                        ko = koc * koc_n + koi
                        for nb in range(NB):
                            nc.tensor.matmul(
                                ptiles[nb][:, :],
                                w_tile[:, ko, nb * P:(nb + 1) * P],
                                a_tile[:, koi, :],
                                start=(ko == 0),
                                stop=(ko == KO - 1),
                            )
                o_tile = opool.tile([P, NB, L], f32, tag="o", name="o_tile",
                                    padded_shape=[P, NB, TB])
                for nb in range(NB):
                    nc.vector.tensor_copy(o_tile[:, nb, :], ptiles[nb][:, off:off + L])
                nc.sync.dma_start(
                    out=ot[:, :, ts:ts + L].transpose([1, 0, 2]),
                    in_=o_tile[:, :, :],
                )
    nc.compile()
    return nc


def kernel(a, b, c, seg_indptr, weight_indices, batch_size, **_):
    T, K = a.shape
    E, N, K2 = b.shape
    assert K == K2
    NS = N // NC

    seg = np.asarray(seg_indptr).astype(np.int64)
    widx_arr = np.asarray(weight_indices).astype(np.int64)
    segs = [(int(seg[e]), int(seg[e + 1]), int(widx_arr[e]))
            for e in range(int(batch_size)) if seg[e + 1] > seg[e]]
    seg_starts = [s for s, _, _ in segs]
    seg_ends = [t for _, t, _ in segs]
    experts = [w for _, _, w in segs]
    EA = len(segs)
    blocks = _token_blocks(seg_starts, seg_ends)

    a = np.ascontiguousarray(a, dtype=np.float32)
    at_np = np.ascontiguousarray(a.T).reshape(K // P, P, T)

    KO = K // P
    in_maps = []
    for j in range(NC):
        w = np.empty((EA, KO, P, NS), dtype=np.float32)
        for ei, e in enumerate(experts):
            # b[e] is [N, K] row-major; out = a @ b[e].T needs W^T = [K, NS]
            w[ei] = np.ascontiguousarray(
                b[e][j * NS:(j + 1) * NS, :].T
            ).reshape(KO, P, NS)
        in_maps.append({"at": at_np, "wt": w})

    nc = _build_program(T, K, NS, EA, blocks)

    import os
    trace = bool(int(os.environ.get("BASS_KERNEL_TRACE", "0")))
    res = run_bass_kernel_spmd(nc, in_maps, list(range(NC)), trace=trace)
    LAST_RESULT["exec_time_ns"] = res.exec_time_ns
    LAST_RESULT["results"] = res

    out_t = np.empty((N, T), dtype=np.float32)
    for j in range(NC):
        out_t[j * NS:(j + 1) * NS] = res.results[j]["ot"].reshape(NS, T)
    return np.ascontiguousarray(out_t.T)



# revision 2
# speedup vs baseline: 1.9045x; 1.9045x over previous
"""Grouped GEMM (MoE routing) Trainium2 kernel.

Strategy: tensor-parallel shard of the output N dim across 8 NeuronCores.
Every core sees all T=8192 tokens and a 512-wide slice of every expert's
weights, so per-core work is identical regardless of segment sizes and a
single SPMD program (with the segment boundaries baked in as compile-time
constants) runs on all 8 cores.

Per core:  out_t[n, t] = sum_k w_t[e(t), k, n] * a_t[k, t]

v2: inputs are cast to bf16 on the host (rel err ~1e-3, far under the
2e-2 gate) which halves HBM traffic, and all DMAs are laid out so each
SBUF partition line is one long contiguous HBM run (32KB for a/w tiles).
a-block loads ride the sync HWDGE queue; weight loads + output stores
ride the scalar HWDGE queue, so the two streams don't serialize.

Matmul mapping: stationary lhsT = w tile [k=128, n=128], moving rhs =
a tile [k=128, tok<=512] in bf16, PSUM out [n=128, tok<=512] fp32,
accumulated over the 32 k-chunks.  Compute floor/core = T*K*NS/(128*128)
cycles @2.4GHz = 437us; DMA ~109MB/core split over 2 queues stays under
that, so the kernel is compute-bound.
"""

import numpy as np
import ml_dtypes

import concourse.bacc as bacc
import concourse.bass as bass
import concourse.mybir as mybir
import concourse.tile as tile
from concourse.bass_utils import run_bass_kernel_spmd

NC = 8          # NeuronCores
P = 128         # partitions
TB = 512        # max token block (PSUM bank = 512 fp32)

BF16 = ml_dtypes.bfloat16

LAST_RESULT = {}


def _token_blocks(seg_starts, seg_ends):
    """Split each segment into even pieces of <=512 tokens."""
    blocks = []  # (tstart, tlen, active_expert_idx)
    for widx, (s, t) in enumerate(zip(seg_starts, seg_ends)):
        ln = t - s
        npieces = max(1, -(-ln // TB))
        base, rem = divmod(ln, npieces)
        p = s
        for i in range(npieces):
            L = base + (1 if i < rem else 0)
            if L > 0:
                blocks.append((p, L, widx))
                p += L
    return blocks


def _build_program(T, K, NS, EA, blocks):
    f32 = mybir.dt.float32
    bf16 = mybir.dt.bfloat16
    KO = K // P
    NB = NS // P

    CTA = sum(KO * L for (_, L, _) in blocks)
    CTO = sum(NB * L for (_, L, _) in blocks)

    nc = bacc.Bacc(None, target_bir_lowering=False)
    ab = nc.declare_dram_parameter("ab", [P, CTA], bf16, isOutput=False)
    wb = nc.declare_dram_parameter("wb", [EA, P, KO, NS], bf16, isOutput=False)
    ot = nc.declare_dram_parameter("ot", [P, CTO], f32, isOutput=True)

    with tile.TileContext(nc) as tc:
        with (
            tc.tile_pool(name="wpool", bufs=2) as wpool,
            tc.tile_pool(name="apool", bufs=2) as apool,
            tc.tile_pool(name="opool", bufs=2) as opool,
            tc.tile_pool(name="psum", bufs=8, space=bass.MemorySpace.PSUM) as psum_pool,
        ):
            cur_widx = -1
            w_tile = None
            off_a = 0
            off_o = 0
            for (ts, L, widx) in blocks:
                if widx != cur_widx:
                    w_tile = wpool.tile([P, KO, NS], bf16, tag="w", name="w_tile")
                    nc.scalar.dma_start(out=w_tile[:, :, :], in_=wb[widx])
                    cur_widx = widx
                a_tile = apool.tile([P, KO * L], bf16, tag="a", name="a_tile",
                                    padded_shape=[P, KO * TB])
                nc.sync.dma_start(out=a_tile[:, :], in_=ab[:, off_a:off_a + KO * L])
                ptiles = [psum_pool.tile([P, L], f32, tag="ps", name=f"ps{nb}",
                                         padded_shape=[P, TB])
                          for nb in range(NB)]
                for ko in range(KO):
                    for nb in range(NB):
                        nc.tensor.matmul(
                            ptiles[nb][:, :],
                            w_tile[:, ko, nb * P:(nb + 1) * P],
                            a_tile[:, ko * L:(ko + 1) * L],
                            start=(ko == 0),
                            stop=(ko == KO - 1),
                        )
                o_tile = opool.tile([P, NB * L], f32, tag="o", name="o_tile",
                                    padded_shape=[P, NB * TB])
                for nb in range(NB):
                    nc.vector.tensor_copy(o_tile[:, nb * L:(nb + 1) * L],
                                          ptiles[nb][:, :])
                nc.scalar.dma_start(out=ot[:, off_o:off_o + NB * L],
                                    in_=o_tile[:, :])
                off_a += KO * L
                off_o += NB * L
    nc.compile()
    return nc


def kernel(a, b, c, seg_indptr, weight_indices, batch_size, **_):
    T, K = a.shape
    E, N, K2 = b.shape
    assert K == K2
    NS = N // NC
    KO = K // P
    NB = NS // P

    seg = np.asarray(seg_indptr).astype(np.int64)
    widx_arr = np.asarray(weight_indices).astype(np.int64)
    segs = [(int(seg[e]), int(seg[e + 1]), int(widx_arr[e]))
            for e in range(int(batch_size)) if seg[e + 1] > seg[e]]
    seg_starts = [s for s, _, _ in segs]
    seg_ends = [t for _, t, _ in segs]
    experts = [w for _, _, w in segs]
    EA = len(segs)
    blocks = _token_blocks(seg_starts, seg_ends)

    # a -> [P, KO, T] bf16 (partition-major k layout), then pack blocks so
    # each block is a [P, KO*L] slab with 32KB-contiguous partition lines.
    a = np.ascontiguousarray(a, dtype=np.float32)
    at_full = a.T.reshape(KO, P, T).transpose(1, 0, 2).astype(BF16)  # [P,KO,T]
    CTA = sum(KO * L for (_, L, _) in blocks)
    ab_np = np.empty((P, CTA), dtype=BF16)
    off = 0
    for (ts, L, _) in blocks:
        ab_np[:, off:off + KO * L] = at_full[:, :, ts:ts + L].reshape(P, KO * L)
        off += KO * L

    # weights: full [E_active, P, KO, N] bf16 once, slice per core.
    wt_full = np.empty((EA, P, KO, N), dtype=BF16)
    for ei, e in enumerate(experts):
        wt_full[ei] = b[e].T.reshape(KO, P, N).transpose(1, 0, 2)

    in_maps = []
    for j in range(NC):
        w = np.ascontiguousarray(wt_full[:, :, :, j * NS:(j + 1) * NS])
        in_maps.append({"ab": ab_np, "wb": w})

    nc = _build_program(T, K, NS, EA, blocks)

    import os
    trace = bool(int(os.environ.get("BASS_KERNEL_TRACE", "0")))
    res = run_bass_kernel_spmd(nc, in_maps, list(range(NC)), trace=trace)
    LAST_RESULT["exec_time_ns"] = res.exec_time_ns
    LAST_RESULT["results"] = res

    out_t = np.empty((N, T), dtype=np.float32)
    for j in range(NC):
        otj = res.results[j]["ot"]  # [P, CTO]
        off = 0
        for (ts, L, _) in blocks:
            blk = otj[:, off:off + NB * L].reshape(P, NB, L)
            out_t[j * NS:(j + 1) * NS, ts:ts + L] = (
                blk.transpose(1, 0, 2).reshape(NS, L))
            off += NB * L
    return np.ascontiguousarray(out_t.T)


# revision 4
# speedup vs baseline: 2.0574x; 1.0803x over previous
"""Grouped GEMM (MoE routing) Trainium2 kernel.

Strategy: tensor-parallel shard of the output N dim across 8 NeuronCores.
Every core sees all T=8192 tokens and a 512-wide slice of every expert's
weights, so per-core work is identical regardless of segment sizes and a
single SPMD program (with the segment boundaries baked in as compile-time
constants) runs on all 8 cores.

Per core:  out_t[n, t] = sum_k w_t[e(t), k, n] * a_t[k, t]

v2: inputs are cast to bf16 on the host (rel err ~1e-3, far under the
2e-2 gate) which halves HBM traffic, and all DMAs are laid out so each
SBUF partition line is one long contiguous HBM run (32KB for a/w tiles).
a-block loads ride the sync HWDGE queue; weight loads + output stores
ride the scalar HWDGE queue, so the two streams don't serialize.

Matmul mapping: stationary lhsT = w tile [k=128, n=128], moving rhs =
a tile [k=128, tok<=512] in bf16, PSUM out [n=128, tok<=512] fp32,
accumulated over the 32 k-chunks.  Compute floor/core = T*K*NS/(128*128)
cycles @2.4GHz = 437us; DMA ~109MB/core split over 2 queues stays under
that, so the kernel is compute-bound.
"""

import numpy as np
import ml_dtypes

import concourse.bacc as bacc
import concourse.bass as bass
import concourse.mybir as mybir
import concourse.tile as tile
from concourse.bass_utils import run_bass_kernel_spmd

NC = 8          # NeuronCores
P = 128         # partitions
TB = 512        # max token block (PSUM bank = 512 fp32)

BF16 = ml_dtypes.bfloat16

LAST_RESULT = {}


def _token_blocks(seg_starts, seg_ends):
    """Split each segment into even pieces of <=512 tokens."""
    blocks = []  # (tstart, tlen, active_expert_idx)
    for widx, (s, t) in enumerate(zip(seg_starts, seg_ends)):
        ln = t - s
        npieces = max(1, -(-ln // TB))
        base, rem = divmod(ln, npieces)
        p = s
        for i in range(npieces):
            L = base + (1 if i < rem else 0)
            if L > 0:
                blocks.append((p, L, widx))
                p += L
    return blocks


WCH = 8         # ko per weight/a DMA chunk (4 chunks of 8KB+/partition)


def _build_program(T, K, NS, EA, blocks):
    f32 = mybir.dt.float32
    bf16 = mybir.dt.bfloat16
    KO = K // P
    NB = NS // P
    NCH = KO // WCH

    CTA = sum(KO * L for (_, L, _) in blocks)
    CTO = sum(NB * L for (_, L, _) in blocks)

    # group consecutive same-expert blocks into runs
    runs = []
    for blk in blocks:
        if runs and runs[-1][0] == blk[2]:
            runs[-1][1].append(blk)
        else:
            runs.append((blk[2], [blk]))

    nc = bacc.Bacc(None, target_bir_lowering=False)
    ab = nc.declare_dram_parameter("ab", [P, CTA], bf16, isOutput=False)
    wb = nc.declare_dram_parameter("wb", [EA, P, KO, NS], bf16, isOutput=False)
    ot = nc.declare_dram_parameter("ot", [P, CTO], f32, isOutput=True)

    with tile.TileContext(nc) as tc:
        with (
            tc.tile_pool(name="wpool", bufs=3) as wpool,
            tc.tile_pool(name="apool", bufs=2) as apool,
            tc.tile_pool(name="opool", bufs=2) as opool,
            tc.tile_pool(name="psum", bufs=8, space=bass.MemorySpace.PSUM) as psum_pool,
        ):
            def load_w(ri):
                widx = runs[ri][0]
                wt = wpool.tile([P, KO, NS], bf16, tag="w", name="w_tile")
                for c in range(NCH):
                    nc.scalar.dma_start(
                        out=wt[:, c * WCH:(c + 1) * WCH, :],
                        in_=wb[widx, :, c * WCH:(c + 1) * WCH, :])
                return wt

            off_a = 0
            off_o = 0
            w_next = load_w(0)
            for ri, (widx, rblocks) in enumerate(runs):
                w_tile = w_next
                for bi, (ts, L, _) in enumerate(rblocks):
                    a_tile = apool.tile([P, KO * L], bf16, tag="a", name="a_tile",
                                        padded_shape=[P, KO * TB])
                    for c in range(NCH):
                        nc.sync.dma_start(
                            out=a_tile[:, c * WCH * L:(c + 1) * WCH * L],
                            in_=ab[:, off_a + c * WCH * L:off_a + (c + 1) * WCH * L])
                    ptiles = [psum_pool.tile([P, L], f32, tag="ps", name=f"ps{nb}",
                                             padded_shape=[P, TB])
                              for nb in range(NB)]
                    for ko in range(KO):
                        for nb in range(NB):
                            nc.tensor.matmul(
                                ptiles[nb][:, :],
                                w_tile[:, ko, nb * P:(nb + 1) * P],
                                a_tile[:, ko * L:(ko + 1) * L],
                                start=(ko == 0),
                                stop=(ko == KO - 1),
                            )
                    o_tile = opool.tile([P, NB * L], f32, tag="o", name="o_tile",
                                        padded_shape=[P, NB * TB])
                    for nb in range(NB):
                        nc.vector.tensor_copy(o_tile[:, nb * L:(nb + 1) * L],
                                              ptiles[nb][:, :])
                        nc.scalar.dma_start(
                            out=ot[:, off_o + nb * L:off_o + (nb + 1) * L],
                            in_=o_tile[:, nb * L:(nb + 1) * L])
                    off_a += KO * L
                    off_o += NB * L
                    if bi == 0 and ri + 1 < len(runs):
                        # prefetch next expert's weights right after this
                        # run's first block so the transfer hides under the
                        # rest of the run
                        w_next = load_w(ri + 1)
    nc.compile()
    return nc


def kernel(a, b, c, seg_indptr, weight_indices, batch_size, **_):
    T, K = a.shape
    E, N, K2 = b.shape
    assert K == K2
    NS = N // NC
    KO = K // P
    NB = NS // P

    seg = np.asarray(seg_indptr).astype(np.int64)
    widx_arr = np.asarray(weight_indices).astype(np.int64)
    segs = [(int(seg[e]), int(seg[e + 1]), int(widx_arr[e]))
            for e in range(int(batch_size)) if seg[e + 1] > seg[e]]
    # process longest segments first: every expert switch is then covered by
    # a long compute run, hiding the next weight load entirely
    segs.sort(key=lambda s: s[0] - s[1])
    seg_starts = [s for s, _, _ in segs]
    seg_ends = [t for _, t, _ in segs]
    experts = [w for _, _, w in segs]
    EA = len(segs)
    blocks = _token_blocks(seg_starts, seg_ends)

    # a -> [P, KO, T] bf16 (partition-major k layout), then pack blocks so
    # each block is a [P, KO*L] slab with 32KB-contiguous partition lines.
    a = np.ascontiguousarray(a, dtype=np.float32)
    at_full = a.T.reshape(KO, P, T).transpose(1, 0, 2).astype(BF16)  # [P,KO,T]
    CTA = sum(KO * L for (_, L, _) in blocks)
    ab_np = np.empty((P, CTA), dtype=BF16)
    off = 0
    for (ts, L, _) in blocks:
        ab_np[:, off:off + KO * L] = at_full[:, :, ts:ts + L].reshape(P, KO * L)
        off += KO * L

    # weights: full [E_active, P, KO, N] bf16 once, slice per core.
    wt_full = np.empty((EA, P, KO, N), dtype=BF16)
    for ei, e in enumerate(experts):
        wt_full[ei] = b[e].T.reshape(KO, P, N).transpose(1, 0, 2)

    in_maps = []
    for j in range(NC):
        w = np.ascontiguousarray(wt_full[:, :, :, j * NS:(j + 1) * NS])
        in_maps.append({"ab": ab_np, "wb": w})

    nc = _build_program(T, K, NS, EA, blocks)

    import os
    trace = bool(int(os.environ.get("BASS_KERNEL_TRACE", "0")))
    res = run_bass_kernel_spmd(nc, in_maps, list(range(NC)), trace=trace)
    LAST_RESULT["exec_time_ns"] = res.exec_time_ns
    LAST_RESULT["results"] = res

    out_t = np.empty((N, T), dtype=np.float32)
    for j in range(NC):
        otj = res.results[j]["ot"]  # [P, CTO]
        off = 0
        for (ts, L, _) in blocks:
            blk = otj[:, off:off + NB * L].reshape(P, NB, L)
            out_t[j * NS:(j + 1) * NS, ts:ts + L] = (
                blk.transpose(1, 0, 2).reshape(NS, L))
            off += NB * L
    return np.ascontiguousarray(out_t.T)


# revision 6
# speedup vs baseline: 2.0799x; 1.0109x over previous
"""Grouped GEMM (MoE routing) Trainium2 kernel.

Strategy: tensor-parallel shard of the output N dim across 8 NeuronCores.
Every core sees all T=8192 tokens and a 512-wide slice of every expert's
weights, so per-core work is identical regardless of segment sizes and a
single SPMD program (with the segment boundaries baked in as compile-time
constants) runs on all 8 cores.

Per core:  out_t[n, t] = sum_k w_t[e(t), k, n] * a_t[k, t]

v2: inputs are cast to bf16 on the host (rel err ~1e-3, far under the
2e-2 gate) which halves HBM traffic, and all DMAs are laid out so each
SBUF partition line is one long contiguous HBM run (32KB for a/w tiles).
a-block loads ride the sync HWDGE queue; weight loads + output stores
ride the scalar HWDGE queue, so the two streams don't serialize.

Matmul mapping: stationary lhsT = w tile [k=128, n=128], moving rhs =
a tile [k=128, tok<=512] in bf16, PSUM out [n=128, tok<=512] fp32,
accumulated over the 32 k-chunks.  Compute floor/core = T*K*NS/(128*128)
cycles @2.4GHz = 437us; DMA ~109MB/core split over 2 queues stays under
that, so the kernel is compute-bound.
"""

import numpy as np
import ml_dtypes

import concourse.bacc as bacc
import concourse.bass as bass
import concourse.mybir as mybir
import concourse.tile as tile
from concourse.bass_utils import run_bass_kernel_spmd

NC = 8          # NeuronCores
P = 128         # partitions
TB = 512        # max token block (PSUM bank = 512 fp32)

BF16 = ml_dtypes.bfloat16

LAST_RESULT = {}


def _token_blocks(seg_starts, seg_ends):
    """Split each segment into even pieces of <=512 tokens."""
    blocks = []  # (tstart, tlen, active_expert_idx)
    for widx, (s, t) in enumerate(zip(seg_starts, seg_ends)):
        ln = t - s
        npieces = max(1, -(-ln // TB))
        base, rem = divmod(ln, npieces)
        p = s
        for i in range(npieces):
            L = base + (1 if i < rem else 0)
            if L > 0:
                blocks.append((p, L, widx))
                p += L
    return blocks


WCH = 8         # ko per weight/a DMA chunk (4 chunks of 8KB+/partition)


def _build_program(T, K, NS, EA, blocks):
    f32 = mybir.dt.float32
    bf16 = mybir.dt.bfloat16
    KO = K // P
    NB = NS // P
    NCH = KO // WCH

    CTA = sum(KO * L for (_, L, _) in blocks)
    CTO = sum(NB * L for (_, L, _) in blocks)

    # group consecutive same-expert blocks into runs
    runs = []
    for blk in blocks:
        if runs and runs[-1][0] == blk[2]:
            runs[-1][1].append(blk)
        else:
            runs.append((blk[2], [blk]))

    nc = bacc.Bacc(None, target_bir_lowering=False)
    ab = nc.declare_dram_parameter("ab", [P, CTA], bf16, isOutput=False)
    wb = nc.declare_dram_parameter("wb", [EA, P, KO, NS], bf16, isOutput=False)
    ot = nc.declare_dram_parameter("ot", [P, CTO], bf16, isOutput=True)

    with tile.TileContext(nc) as tc:
        with (
            tc.tile_pool(name="wpool", bufs=3) as wpool,
            tc.tile_pool(name="apool", bufs=3) as apool,
            tc.tile_pool(name="opool", bufs=2) as opool,
            tc.tile_pool(name="psum", bufs=8, space=bass.MemorySpace.PSUM) as psum_pool,
        ):
            def load_w_chunk(wt, widx, c):
                nc.scalar.dma_start(
                    out=wt[:, c * WCH:(c + 1) * WCH, :],
                    in_=wb[widx, :, c * WCH:(c + 1) * WCH, :])

            off_a = 0
            off_o = 0
            w_next = wpool.tile([P, KO, NS], bf16, tag="w", name="w_tile")
            for c in range(NCH):
                load_w_chunk(w_next, runs[0][0], c)
            for ri, (widx, rblocks) in enumerate(runs):
                w_tile = w_next
                nbk = len(rblocks)
                if ri + 1 < len(runs):
                    w_next = wpool.tile([P, KO, NS], bf16, tag="w", name="w_tile")
                for bi, (ts, L, _) in enumerate(rblocks):
                    a_tile = apool.tile([P, KO * L], bf16, tag="a", name="a_tile",
                                        padded_shape=[P, KO * TB])
                    for c in range(NCH):
                        nc.sync.dma_start(
                            out=a_tile[:, c * WCH * L:(c + 1) * WCH * L],
                            in_=ab[:, off_a + c * WCH * L:off_a + (c + 1) * WCH * L])
                    ptiles = [psum_pool.tile([P, L], f32, tag="ps", name=f"ps{nb}",
                                             padded_shape=[P, TB])
                              for nb in range(NB)]
                    for ko in range(KO):
                        for nb in range(NB):
                            nc.tensor.matmul(
                                ptiles[nb][:, :],
                                w_tile[:, ko, nb * P:(nb + 1) * P],
                                a_tile[:, ko * L:(ko + 1) * L],
                                start=(ko == 0),
                                stop=(ko == KO - 1),
                            )
                    o_tile = opool.tile([P, NB * L], bf16, tag="o", name="o_tile",
                                        padded_shape=[P, NB * TB])
                    for nb in range(NB):
                        nc.vector.tensor_copy(o_tile[:, nb * L:(nb + 1) * L],
                                              ptiles[nb][:, :])
                    nc.scalar.dma_start(out=ot[:, off_o:off_o + NB * L],
                                        in_=o_tile[:, :])
                    off_a += KO * L
                    off_o += NB * L
                    # pace the next expert's weight chunks across this run's
                    # blocks so the prefetch never bursts against the a-stream
                    if ri + 1 < len(runs):
                        c0 = bi * NCH // nbk
                        c1 = (bi + 1) * NCH // nbk
                        for c in range(c0, c1):
                            load_w_chunk(w_next, runs[ri + 1][0], c)
    nc.compile()
    return nc


def kernel(a, b, c, seg_indptr, weight_indices, batch_size, **_):
    T, K = a.shape
    E, N, K2 = b.shape
    assert K == K2
    NS = N // NC
    KO = K // P
    NB = NS // P

    seg = np.asarray(seg_indptr).astype(np.int64)
    widx_arr = np.asarray(weight_indices).astype(np.int64)
    segs = [(int(seg[e]), int(seg[e + 1]), int(widx_arr[e]))
            for e in range(int(batch_size)) if seg[e + 1] > seg[e]]
    # process longest segments first: every expert switch is then covered by
    # a long compute run, hiding the next weight load entirely
    segs.sort(key=lambda s: s[0] - s[1])
    seg_starts = [s for s, _, _ in segs]
    seg_ends = [t for _, t, _ in segs]
    experts = [w for _, _, w in segs]
    EA = len(segs)
    blocks = _token_blocks(seg_starts, seg_ends)

    # a -> [P, KO, T] bf16 (partition-major k layout), then pack blocks so
    # each block is a [P, KO*L] slab with 32KB-contiguous partition lines.
    a = np.ascontiguousarray(a, dtype=np.float32)
    at_full = a.T.reshape(KO, P, T).transpose(1, 0, 2).astype(BF16)  # [P,KO,T]
    CTA = sum(KO * L for (_, L, _) in blocks)
    ab_np = np.empty((P, CTA), dtype=BF16)
    off = 0
    for (ts, L, _) in blocks:
        ab_np[:, off:off + KO * L] = at_full[:, :, ts:ts + L].reshape(P, KO * L)
        off += KO * L

    # weights: full [E_active, P, KO, N] bf16 once, slice per core.
    wt_full = np.empty((EA, P, KO, N), dtype=BF16)
    for ei, e in enumerate(experts):
        wt_full[ei] = b[e].T.reshape(KO, P, N).transpose(1, 0, 2)

    in_maps = []
    for j in range(NC):
        w = np.ascontiguousarray(wt_full[:, :, :, j * NS:(j + 1) * NS])
        in_maps.append({"ab": ab_np, "wb": w})

    nc = _build_program(T, K, NS, EA, blocks)

    import os
    trace = bool(int(os.environ.get("BASS_KERNEL_TRACE", "0")))
    res = run_bass_kernel_spmd(nc, in_maps, list(range(NC)), trace=trace)
    LAST_RESULT["exec_time_ns"] = res.exec_time_ns
    LAST_RESULT["results"] = res

    out_t = np.empty((N, T), dtype=np.float32)
    for j in range(NC):
        otj = np.asarray(res.results[j]["ot"]).astype(np.float32)  # [P, CTO]
        off = 0
        for (ts, L, _) in blocks:
            blk = otj[:, off:off + NB * L].reshape(P, NB, L)
            out_t[j * NS:(j + 1) * NS, ts:ts + L] = (
                blk.transpose(1, 0, 2).reshape(NS, L))
            off += NB * L
    return np.ascontiguousarray(out_t.T)


# revision 8
# speedup vs baseline: 2.1144x; 1.0166x over previous
"""Grouped GEMM (MoE routing) Trainium2 kernel.

Strategy: tensor-parallel shard of the output N dim across 8 NeuronCores.
Every core sees all T=8192 tokens and a 512-wide slice of every expert's
weights, so per-core work is identical regardless of segment sizes and a
single SPMD program (with the segment boundaries baked in as compile-time
constants) runs on all 8 cores.

Per core:  out_t[n, t] = sum_k w_t[e(t), k, n] * a_t[k, t]

v2: inputs are cast to bf16 on the host (rel err ~1e-3, far under the
2e-2 gate) which halves HBM traffic, and all DMAs are laid out so each
SBUF partition line is one long contiguous HBM run (32KB for a/w tiles).
a-block loads ride the sync HWDGE queue; weight loads + output stores
ride the scalar HWDGE queue, so the two streams don't serialize.

Matmul mapping: stationary lhsT = w tile [k=128, n=128], moving rhs =
a tile [k=128, tok<=512] in bf16, PSUM out [n=128, tok<=512] fp32,
accumulated over the 32 k-chunks.  Compute floor/core = T*K*NS/(128*128)
cycles @2.4GHz = 437us; DMA ~109MB/core split over 2 queues stays under
that, so the kernel is compute-bound.
"""

import numpy as np
import ml_dtypes

import concourse.bacc as bacc
import concourse.bass as bass
import concourse.mybir as mybir
import concourse.tile as tile
from concourse.bass_utils import run_bass_kernel_spmd

NC = 8          # NeuronCores
P = 128         # partitions
TB = 512        # max token block (PSUM bank = 512 fp32)

BF16 = ml_dtypes.bfloat16

LAST_RESULT = {}


def _token_blocks(seg_starts, seg_ends):
    """Split each segment into even pieces of <=512 tokens."""
    blocks = []  # (tstart, tlen, active_expert_idx)
    for widx, (s, t) in enumerate(zip(seg_starts, seg_ends)):
        ln = t - s
        npieces = max(1, -(-ln // TB))
        base, rem = divmod(ln, npieces)
        p = s
        for i in range(npieces):
            L = base + (1 if i < rem else 0)
            if L > 0:
                blocks.append((p, L, widx))
                p += L
    return blocks


WCH = 8         # ko per weight/a DMA chunk (4 chunks of 8KB+/partition)


def _build_program(T, K, NS, EA, blocks):
    f32 = mybir.dt.float32
    bf16 = mybir.dt.bfloat16
    KO = K // P
    NB = NS // P
    NCH = KO // WCH

    CTA = sum(KO * L for (_, L, _) in blocks)
    CTO = sum(NB * L for (_, L, _) in blocks)

    # group consecutive same-expert blocks into runs
    runs = []
    for blk in blocks:
        if runs and runs[-1][0] == blk[2]:
            runs[-1][1].append(blk)
        else:
            runs.append((blk[2], [blk]))

    nc = bacc.Bacc(None, target_bir_lowering=False)
    ab = nc.declare_dram_parameter("ab", [P, CTA], bf16, isOutput=False)
    wb = nc.declare_dram_parameter("wb", [EA, P, KO, NS], bf16, isOutput=False)
    ot = nc.declare_dram_parameter("ot", [P, CTO], bf16, isOutput=True)

    with tile.TileContext(nc) as tc:
        with (
            tc.tile_pool(name="wpool", bufs=3) as wpool,
            tc.tile_pool(name="apool", bufs=3) as apool,
            tc.tile_pool(name="opool", bufs=2) as opool,
            tc.tile_pool(name="psum", bufs=8, space=bass.MemorySpace.PSUM) as psum_pool,
        ):
            def load_w_chunk(wt, widx, c, step=WCH):
                nc.scalar.dma_start(
                    out=wt[:, c * step:(c + 1) * step, :],
                    in_=wb[widx, :, c * step:(c + 1) * step, :])

            # PE pre-warm: ~40 dummy matmuls on scratch SBUF with no DMA
            # deps run during the initial load, so the HAM clock-gate opens
            # (1.2->2.4GHz takes ~3.4us of sustained PE work) before the
            # first real matmul issues.
            warm_w = wpool.tile([P, P], bf16, tag="warm", name="warm_w")
            warm_a = apool.tile([P, 256], bf16, tag="warm", name="warm_a")
            nc.vector.memset(warm_w[:, :], 0)
            nc.vector.memset(warm_a[:, :], 0)
            warm_ps = psum_pool.tile([P, 256], f32, tag="ps", name="warm_ps",
                                     padded_shape=[P, TB])
            for _ in range(40):
                nc.tensor.matmul(warm_ps[:, :], warm_w[:, :], warm_a[:, :],
                                 start=True, stop=True)

            off_a = 0
            off_o = 0
            w_next = wpool.tile([P, KO, NS], bf16, tag="w", name="w_tile")
            for c in range(2 * NCH):
                load_w_chunk(w_next, runs[0][0], c, step=WCH // 2)
            for ri, (widx, rblocks) in enumerate(runs):
                w_tile = w_next
                nbk = len(rblocks)
                if ri + 1 < len(runs):
                    w_next = wpool.tile([P, KO, NS], bf16, tag="w", name="w_tile")
                for bi, (ts, L, _) in enumerate(rblocks):
                    a_tile = apool.tile([P, KO * L], bf16, tag="a", name="a_tile",
                                        padded_shape=[P, KO * TB])
                    ach = WCH // 2 if (ri == 0 and bi == 0) else WCH
                    for c in range(KO // ach):
                        nc.sync.dma_start(
                            out=a_tile[:, c * ach * L:(c + 1) * ach * L],
                            in_=ab[:, off_a + c * ach * L:off_a + (c + 1) * ach * L])
                    ptiles = [psum_pool.tile([P, L], f32, tag="ps", name=f"ps{nb}",
                                             padded_shape=[P, TB])
                              for nb in range(NB)]
                    for ko in range(KO):
                        for nb in range(NB):
                            nc.tensor.matmul(
                                ptiles[nb][:, :],
                                w_tile[:, ko, nb * P:(nb + 1) * P],
                                a_tile[:, ko * L:(ko + 1) * L],
                                start=(ko == 0),
                                stop=(ko == KO - 1),
                            )
                    o_tile = opool.tile([P, NB * L], bf16, tag="o", name="o_tile",
                                        padded_shape=[P, NB * TB])
                    for nb in range(NB):
                        nc.vector.tensor_copy(o_tile[:, nb * L:(nb + 1) * L],
                                              ptiles[nb][:, :])
                    nc.scalar.dma_start(out=ot[:, off_o:off_o + NB * L],
                                        in_=o_tile[:, :])
                    off_a += KO * L
                    off_o += NB * L
                    # pace the next expert's weight chunks across this run's
                    # blocks so the prefetch never bursts against the a-stream
                    if ri + 1 < len(runs):
                        c0 = bi * NCH // nbk
                        c1 = (bi + 1) * NCH // nbk
                        for c in range(c0, c1):
                            load_w_chunk(w_next, runs[ri + 1][0], c)
    nc.compile()
    return nc


def kernel(a, b, c, seg_indptr, weight_indices, batch_size, **_):
    T, K = a.shape
    E, N, K2 = b.shape
    assert K == K2
    NS = N // NC
    KO = K // P
    NB = NS // P

    seg = np.asarray(seg_indptr).astype(np.int64)
    widx_arr = np.asarray(weight_indices).astype(np.int64)
    segs = [(int(seg[e]), int(seg[e + 1]), int(widx_arr[e]))
            for e in range(int(batch_size)) if seg[e + 1] > seg[e]]
    # process longest segments first: every expert switch is then covered by
    # a long compute run, hiding the next weight load entirely
    segs.sort(key=lambda s: s[0] - s[1])
    seg_starts = [s for s, _, _ in segs]
    seg_ends = [t for _, t, _ in segs]
    experts = [w for _, _, w in segs]
    EA = len(segs)
    blocks = _token_blocks(seg_starts, seg_ends)

    # a -> [P, KO, T] bf16 (partition-major k layout), then pack blocks so
    # each block is a [P, KO*L] slab with 32KB-contiguous partition lines.
    a = np.ascontiguousarray(a, dtype=np.float32)
    at_full = a.T.reshape(KO, P, T).transpose(1, 0, 2).astype(BF16)  # [P,KO,T]
    CTA = sum(KO * L for (_, L, _) in blocks)
    ab_np = np.empty((P, CTA), dtype=BF16)
    off = 0
    for (ts, L, _) in blocks:
        ab_np[:, off:off + KO * L] = at_full[:, :, ts:ts + L].reshape(P, KO * L)
        off += KO * L

    # weights: full [E_active, P, KO, N] bf16 once, slice per core.
    wt_full = np.empty((EA, P, KO, N), dtype=BF16)
    for ei, e in enumerate(experts):
        wt_full[ei] = b[e].T.reshape(KO, P, N).transpose(1, 0, 2)

    in_maps = []
    for j in range(NC):
        w = np.ascontiguousarray(wt_full[:, :, :, j * NS:(j + 1) * NS])
        in_maps.append({"ab": ab_np, "wb": w})

    nc = _build_program(T, K, NS, EA, blocks)

    import os
    trace = bool(int(os.environ.get("BASS_KERNEL_TRACE", "0")))
    res = run_bass_kernel_spmd(nc, in_maps, list(range(NC)), trace=trace)
    LAST_RESULT["exec_time_ns"] = res.exec_time_ns
    LAST_RESULT["results"] = res

    out_t = np.empty((N, T), dtype=np.float32)
    for j in range(NC):
        otj = np.asarray(res.results[j]["ot"]).astype(np.float32)  # [P, CTO]
        off = 0
        for (ts, L, _) in blocks:
            blk = otj[:, off:off + NB * L].reshape(P, NB, L)
            out_t[j * NS:(j + 1) * NS, ts:ts + L] = (
                blk.transpose(1, 0, 2).reshape(NS, L))
            off += NB * L
    return np.ascontiguousarray(out_t.T)
